# revision 28
# baseline (speedup 1.0000x reference)
"""MobilityGNNLayer Trainium2 kernel (8 NeuronCores, SPMD, no collectives).

Sharding: 1D partition of the destination axis (columns of mobility_matrix).
Core c owns destination nodes i in [c*1024, (c+1)*1024).

Math (validated: max rel err 3.9e-3 vs the fp32 reference, gate 2e-2):
  reference: A = M/(colsum+eps); Wm = A masked at 1e-6; T = X@W_in + b_in;
  agg = (Wm^T T)/(sum Wm + eps); out = LN(agg@W_out + b_out + X).
  Everything except the big SpMM folds into host-side preprocessing:
    - exact threshold mask applied on host (m > 1e-6*(colsum+eps));
    - weights normalized on host: w = Mm/(wsum + eps') so no on-device
      division or weight-sum column is needed;
    - row-scalar division commutes with the right-matmul, so W_in@W_out
      folds into the features: U = X @ (W_in@W_out);
    - xrb = X[shard] + (b_in@W_out + b_out) is the fp32 residual.
  Device per core:  G[i,:] = sum_j w_ji * U[j,:]  (one fp16 matmul stream,
  8 row-blocks x 64 k-tiles, moving dim 256), then per block
  y = G*(1/64) + xrb;  out = LN(y)  on DVE/ACT only.
  Weights are host-scaled by 64 so all fp16 weight values are normal
  (w ~ 1e-6..2.5e-4 would be subnormal; FTZ hardware would zero 25% of
  them). U stays unscaled (validated FTZ-safe).

Precision: fp16 in / fp32 PSUM accumulate. M,U quantization noise is
damped ~80x by the weighted mean and the agg branch is ~0.6% of the
residual, so fp16 lands at 3.9e-3 (bf16 fails at 3.8e-2, fp8 at 0.63).

Schedule (true ridge: PE ~56 us ~= DMA ~55 us per core, so every PE
idle second is a second on the total; the stream must keep the PE fed
from the first microsecond to the last):
  - M is packed IB-MAJOR per partition ([ib][jt][128] fp16) and
    delivered in two regimes on ONE sync-queue stream (a single
    sequential HBM stream sustains peak per-core bandwidth):
      bands jt 0-48: [U chunk | all 8 blocks' M slice] per band --
        each 1 MiB of U unlocks 16 j-tiles x 8 blocks of matmuls
        (~13.7 us PE per ~12.8 us DMA), so the PE never starves while
        the replicated U loads;
      tails jt 48-64: per-block 0.5 MiB pieces -- block k's
        accumulation completes ~1.7 us after block k-1's, so each
        epilogue (residual add, LayerNorm, store) hides under the next
        block's matmuls and the chains never pile up on DVE. Only the
        last block's ~3 us chain is exposed.
  - Descriptor runs stay >= 4 KB/partition (2 KB runs measured 25%
    slower). Output stores go on the scalar queue so they never block
    the M stream.
  - A short burst of warmup matmuls on a zeroed tile holds the PE HAM
    clock-gate at 2.4 GHz through the DMA fill (cold PE runs at 1.2).
"""

import numpy as np

import concourse.bass as bass
import concourse.mybir as mybir
import concourse.tile as tile
from concourse import bacc
from concourse.bass import ts
from concourse.bass_utils import run_bass_kernel_spmd

F32 = mybir.dt.float32
F16 = mybir.dt.float16
AF = mybir.ActivationFunctionType
ALU = mybir.AluOpType

N, D, NCORES = 8192, 256, 8
P = 128
EPS = 1e-8
THR = 1e-6
LN_EPS = 1e-5
WSCALE = 64.0   # host premultiplier keeping fp16 weights in normal range
NWARM = 16      # warmup matmuls bridging the pre-stream PE idle window


def build_program(n=N, d=D, ncores=NCORES, ln_affine=False):
    """Build + compile the SPMD Bass program (per-core column shard)."""
    s = n // ncores          # shard width (destination nodes per core)
    njt = n // P             # contraction tiles
    nib = s // P             # output row-blocks per core

    npair = nib // 2

    nc = bacc.Bacc("TRN2", target_bir_lowering=False, debug=False,
                   num_devices=ncores)
    # m: block-PAIR-major pack; per partition: [pair][jt][256] fp16 --
    # doubles every descriptor run vs per-block packing (8 KB at 16-jt
    # pieces) without changing PE sequencing or epilogue stagger
    m_d = nc.dram_tensor("m_pk", [P, npair * njt * 2 * P], F16,
                         kind="ExternalInput")
    u_d = nc.dram_tensor("u_pk", [P, njt * d], F16, kind="ExternalInput")
    xrb_d = nc.dram_tensor("xrb", [P, nib * d], F32, kind="ExternalInput")
    ln_s = nc.dram_tensor("ln_s", [1, d], F32, kind="ExternalInput")
    ln_b = nc.dram_tensor("ln_b", [1, d], F32, kind="ExternalInput")
    out = nc.dram_tensor("out_shard", [s, d], F32, kind="ExternalOutput")

    with tile.TileContext(nc) as tc:
        with (
            tc.tile_pool(name="const", bufs=1) as const,
            tc.tile_pool(name="mpool", bufs=1) as mpool,
            tc.tile_pool(name="work", bufs=2) as work,
            tc.tile_pool(name="pp", bufs=1, space="PSUM") as pp,
        ):
            eps_t = const.tile([P, 1], F32)
            nc.vector.memset(eps_t[:], LN_EPS)
            if ln_affine:
                lns_bc = const.tile([P, d], F32)
                nc.scalar.dma_start(lns_bc[:], ln_s[:].to_broadcast((P, d)))
                lnb_bc = const.tile([P, d], F32)
                nc.scalar.dma_start(lnb_bc[:], ln_b[:].to_broadcast((P, d)))

            u = const.tile([P, njt, d], F16)
            xrb = const.tile([P, nib, d], F32)
            g = [pp.tile([P, d], F32, tag=f"g{ib}", name=f"g{ib}")
                 for ib in range(nib)]
            mb = [mpool.tile([P, njt, 2 * P], F16, tag=f"mb{p}",
                             name=f"mb{p}") for p in range(npair)]

            # ---- PE warmup: keep the HAM clock-gate open while the DMA
            # stream fills. Zeroed operands into g[0] as complete start/
            # stop groups ahead of the real accumulation; never read. ----
            warm = const.tile([P, d], F16)
            nc.vector.memset(warm[:], 0.0)
            for _ in range(NWARM):
                nc.tensor.matmul(g[0][:], lhsT=warm[:, 0:P], rhs=warm[:],
                                 start=True, stop=True)

            def emit_u(lo, hi):
                nc.sync.dma_start(u[:, lo:hi, :], u_d[:, lo * d:hi * d])

            def emit_m(p, lo, hi):      # j-tiles [lo, hi) of block-pair p
                base = p * njt * 2 * P
                nc.sync.dma_start(
                    mb[p][:, lo:hi, :],
                    m_d[:, base + lo * 2 * P:base + hi * 2 * P])

            def emit_mms(ib, lo, hi):
                half = (ib % 2) * P
                for jt in range(lo, hi):
                    nc.tensor.matmul(
                        g[ib][:],
                        lhsT=mb[ib // 2][:, jt, half:half + P],
                        rhs=u[:, jt, :],
                        start=(jt == 0),
                        stop=(jt == njt - 1))

            # fine bands early (PE fed from the first microsecond), wide
            # bands later (>=4KB descriptor runs); per-block tails sized
            # so completions pitch ~1.4us >= the epilogue's DVE content.
            BANDS = [(0, 16), (16, 32), (32, 48)]
            TAIL = (48, njt)
            for bi, (lo, hi) in enumerate(BANDS):
                if bi == 0:   # split tiny head pieces: first matmul early
                    emit_u(0, 1)
                    emit_m(0, 0, 1)
                    emit_u(1, 8)
                    emit_m(0, 1, 8)
                    emit_u(8, hi)
                    emit_m(0, 8, hi)
                    for p in range(1, npair):
                        emit_m(p, 0, 8)
                        emit_m(p, 8, hi)
                else:
                    emit_u(lo, hi)
                    for p in range(npair):
                        emit_m(p, lo, hi)
                if bi == 1:   # xrb rides mid-stream, due by 1st epilogue
                    nc.sync.dma_start(xrb[:], xrb_d[:])
                for ib in range(nib):
                    emit_mms(ib, lo, hi)

            emit_u(TAIL[0], TAIL[1])
            for p in range(npair):
                emit_m(p, TAIL[0], TAIL[1])

            for ib in range(nib):
                emit_mms(ib, TAIL[0], TAIL[1])

                # ---- epilogue: y = G/WSCALE + xrb, then LayerNorm ----
                y = work.tile([P, d], F32, tag=f"y{ib}", bufs=1,
                              name=f"y{ib}")
                nc.vector.scalar_tensor_tensor(
                    y[:], in0=g[ib][:], scalar=1.0 / WSCALE,
                    in1=xrb[:, ib, :], op0=ALU.mult, op1=ALU.add)

                # LN stats: bn_stats halves -> bn_aggr [mean, var];
                # rstd = rsqrt(var + eps) on ACT; bias = -mean * rstd.
                st6 = work.tile([P, 6], F32, tag=f"st6_{ib}", bufs=1,
                                name=f"st6_{ib}")
                nc.vector.bn_stats(st6[:], y[:])
                mv = work.tile([P, 2], F32, tag=f"mv{ib}", bufs=1,
                               name=f"mv{ib}")
                nc.vector.bn_aggr(mv[:], st6[:])
                sc = work.tile([P, 3], F32, tag=f"sc{ib}", bufs=1,
                               name=f"sc{ib}")
                rstd, bln, stdv = sc[:, 0:1], sc[:, 1:2], sc[:, 2:3]
                nc.scalar.activation(stdv, mv[:, 1:2], AF.Sqrt,
                                     bias=eps_t[:], scale=1.0)
                nc.vector.reciprocal(rstd, stdv)
                nc.vector.scalar_tensor_tensor(
                    bln, in0=mv[:, 0:1], scalar=-1.0, in1=rstd,
                    op0=ALU.mult, op1=ALU.mult)

                yn = work.tile([P, d], F32, tag=f"yn{ib}", bufs=1,
                               name=f"yn{ib}")
                if ib == nib - 1 and not ln_affine:
                    # last block is the exposed tail: normalize halves on
                    # ACT and DVE concurrently, each feeding its store on
                    # a DIFFERENT queue so the two stores overlap
                    nc.scalar.activation(yn[:, 0:d // 2], y[:, 0:d // 2],
                                         AF.Identity, bias=bln, scale=rstd)
                    nc.vector.tensor_scalar(
                        yn[:, d // 2:d], y[:, d // 2:d], rstd, bln,
                        op0=ALU.mult, op1=ALU.add)
                    nc.scalar.dma_start(out[ts(ib, P), 0:d // 2],
                                        yn[:, 0:d // 2])
                    nc.sync.dma_start(out[ts(ib, P), d // 2:d],
                                      yn[:, d // 2:d])
                    continue
                # normalize on ACT: keeps per-chain DVE content below the
                # tail pitch so chains never queue up behind each other
                nc.scalar.activation(yn[:], y[:], AF.Identity,
                                     bias=bln, scale=rstd)
                res = yn
                if ln_affine:
                    t1 = work.tile([P, d], F32, tag=f"t1_{ib}", bufs=1,
                                   name=f"t1_{ib}")
                    nc.vector.tensor_mul(t1[:], yn[:], lns_bc[:])
                    t2 = work.tile([P, d], F32, tag=f"t2_{ib}", bufs=1,
                                   name=f"t2_{ib}")
                    nc.vector.tensor_add(t2[:], t1[:], lnb_bc[:])
                    res = t2
                # scalar queue: stores must never block the M stream
                nc.scalar.dma_start(out[ts(ib, P), :], res[:])

    nc.compile()
    return nc


_cache = {}


def _get_program(ln_affine):
    if ln_affine not in _cache:
        _cache[ln_affine] = build_program(ln_affine=ln_affine)
    return _cache[ln_affine]


def _pack(a, blocks, row_len):
    """[blocks*128, row_len] -> [128, blocks*row_len] with logical row
    blk*128+p at (p, blk*row_len)."""
    return np.ascontiguousarray(
        a.reshape(blocks, P, row_len).transpose(1, 0, 2).reshape(
            P, blocks * row_len))


def prepare_inputs(node_features, mobility_matrix, W_in, b_in, W_out, b_out,
                   ln_scale, ln_bias):
    x = np.asarray(node_features, dtype=np.float32)
    m = np.asarray(mobility_matrix, dtype=np.float32)
    w_in = np.asarray(W_in, dtype=np.float64)
    b_in_ = np.asarray(b_in, dtype=np.float64)
    w_out = np.asarray(W_out, dtype=np.float64)
    b_out_ = np.asarray(b_out, dtype=np.float64)
    lns = np.asarray(ln_scale, dtype=np.float32)
    lnb = np.asarray(ln_bias, dtype=np.float32)

    w_c = w_in @ w_out
    bias_c = (b_in_ @ w_out + b_out_).astype(np.float32)
    ln_affine = not (np.all(lns == 1.0) and np.all(lnb == 0.0))

    # exact threshold mask + host normalization, premultiplied by WSCALE
    colsum = m.sum(axis=0, dtype=np.float64)
    mm = np.where(m > (THR * (colsum + EPS))[None, :].astype(np.float32),
                  m, np.float32(0.0))
    wsum = mm.sum(axis=0, dtype=np.float64)
    col_scale = (WSCALE / (wsum + EPS * (colsum + EPS))).astype(np.float32)
    mh = (mm * col_scale[None, :]).astype(np.float16)
    del mm

    u16 = (x.astype(np.float64) @ w_c).astype(np.float16)
    u_pk = _pack(u16, N // P, D)

    s = N // NCORES
    npair = s // P // 2
    in_maps = []
    for c in range(NCORES):
        # block-pair-major pack: per partition [pair][jt][256]
        m_pk = np.concatenate(
            [_pack(mh[:, c * s + 2 * p * P:c * s + 2 * (p + 1) * P],
                   N // P, 2 * P)
             for p in range(npair)], axis=1)
        in_maps.append({
            "m_pk": np.ascontiguousarray(m_pk),
            "u_pk": u_pk,
            "xrb": _pack(x[c * s:(c + 1) * s] + bias_c, s // P, D),
            "ln_s": lns.reshape(1, D),
            "ln_b": lnb.reshape(1, D),
        })
    return in_maps, ln_affine


def run(in_maps, ln_affine, **kwargs):
    nc = _get_program(ln_affine)
    return run_bass_kernel_spmd(nc, in_maps, core_ids=list(range(NCORES)),
                                **kwargs)


def kernel(**inputs) -> np.ndarray:
    in_maps, ln_affine = prepare_inputs(**inputs)
    res = run(in_maps, ln_affine)
    return np.concatenate([res.results[c]["out_shard"]
                           for c in range(NCORES)], axis=0)


# revision 34
# speedup vs baseline: 1.1273x; 1.1273x over previous
"""MobilityGNNLayer Trainium2 kernel (8 NeuronCores, SPMD, no collectives).

Sharding: 1D partition of the destination axis (columns of mobility_matrix).
Core c owns destination nodes i in [c*1024, (c+1)*1024).

Math (validated: max rel err 3.9e-3 vs the fp32 reference, gate 2e-2):
  reference: A = M/(colsum+eps); Wm = A masked at 1e-6; T = X@W_in + b_in;
  agg = (Wm^T T)/(sum Wm + eps); out = LN(agg@W_out + b_out + X).
  Everything except the big SpMM folds into host-side preprocessing:
    - exact threshold mask applied on host (m > 1e-6*(colsum+eps));
    - weights normalized on host: w = Mm/(wsum + eps') so no on-device
      division or weight-sum column is needed;
    - row-scalar division commutes with the right-matmul, so W_in@W_out
      folds into the features: U = X @ (W_in@W_out);
    - xrb = X[shard] + (b_in@W_out + b_out) is the fp32 residual.
  Device per core:  G[i,:] = sum_j w_ji * U[j,:]  (one fp16 matmul stream,
  8 row-blocks x 64 k-tiles, moving dim 256), then per block
  y = G*(1/64) + xrb;  out = LN(y)  on DVE/ACT only.
  Weights are host-scaled by 64 so all fp16 weight values are normal
  (w ~ 1e-6..2.5e-4 would be subnormal; FTZ hardware would zero 25% of
  them). U stays unscaled (validated FTZ-safe).

Precision: fp16 in / fp32 PSUM accumulate. M,U quantization noise is
damped ~80x by the weighted mean and the agg branch is ~0.6% of the
residual, so fp16 lands at 3.9e-3 (bf16 fails at 3.8e-2, fp8 at 0.63).

Schedule (true ridge: PE ~56 us ~= DMA ~55 us per core, so every PE
idle second is a second on the total; the stream must keep the PE fed
from the first microsecond to the last):
  - M is packed IB-MAJOR per partition ([ib][jt][128] fp16) and
    delivered in two regimes on ONE sync-queue stream (a single
    sequential HBM stream sustains peak per-core bandwidth):
      bands jt 0-48: [U chunk | all 8 blocks' M slice] per band --
        each 1 MiB of U unlocks 16 j-tiles x 8 blocks of matmuls
        (~13.7 us PE per ~12.8 us DMA), so the PE never starves while
        the replicated U loads;
      tails jt 48-64: per-block 0.5 MiB pieces -- block k's
        accumulation completes ~1.7 us after block k-1's, so each
        epilogue (residual add, LayerNorm, store) hides under the next
        block's matmuls and the chains never pile up on DVE. Only the
        last block's ~3 us chain is exposed.
  - Descriptor runs stay >= 4 KB/partition (2 KB runs measured 25%
    slower). Output stores go on the scalar queue so they never block
    the M stream.
  - A short burst of warmup matmuls on a zeroed tile holds the PE HAM
    clock-gate at 2.4 GHz through the DMA fill (cold PE runs at 1.2).
"""

import numpy as np

import concourse.bass as bass
import concourse.mybir as mybir
import concourse.tile as tile
from concourse import bacc
from concourse.bass import ts
from concourse.bass_utils import run_bass_kernel_spmd

F32 = mybir.dt.float32
F16 = mybir.dt.float16
AF = mybir.ActivationFunctionType
ALU = mybir.AluOpType

N, D, NCORES = 8192, 256, 8
P = 128
EPS = 1e-8
THR = 1e-6
LN_EPS = 1e-5
WSCALE = 64.0   # host premultiplier keeping fp16 weights in normal range
NWARM = 16      # warmup matmuls bridging the pre-stream PE idle window


def build_program(n=N, d=D, ncores=NCORES, ln_affine=False):
    """Build + compile the SPMD Bass program (per-core column shard)."""
    s = n // ncores          # shard width (destination nodes per core)
    njt = n // P             # contraction tiles
    nib = s // P             # output row-blocks per core

    nc = bacc.Bacc("TRN2", target_bir_lowering=False, debug=False,
                   num_devices=ncores)
    # m: ib-major pack; per partition: [ib][jt][128] fp16. (NOTE: a
    # block-pair [pair][jt][256] pack was tried for 8KB descriptor runs
    # but the strided lhsT slices un-hide LDWEIGHTS: +18ns per matmul.)
    m_d = nc.dram_tensor("m_pk", [P, nib * njt * P], F16,
                         kind="ExternalInput")
    u_d = nc.dram_tensor("u_pk", [P, njt * d], F16, kind="ExternalInput")
    xrb_d = nc.dram_tensor("xrb", [P, nib * d], F32, kind="ExternalInput")
    ln_s = nc.dram_tensor("ln_s", [1, d], F32, kind="ExternalInput")
    ln_b = nc.dram_tensor("ln_b", [1, d], F32, kind="ExternalInput")
    out = nc.dram_tensor("out_shard", [s, d], F32, kind="ExternalOutput")

    with tile.TileContext(nc) as tc:
        with (
            tc.tile_pool(name="const", bufs=1) as const,
            tc.tile_pool(name="mpool", bufs=1) as mpool,
            tc.tile_pool(name="work", bufs=2) as work,
            tc.tile_pool(name="pp", bufs=1, space="PSUM") as pp,
        ):
            eps_t = const.tile([P, 1], F32)
            nc.vector.memset(eps_t[:], LN_EPS)
            if ln_affine:
                lns_bc = const.tile([P, d], F32)
                nc.scalar.dma_start(lns_bc[:], ln_s[:].to_broadcast((P, d)))
                lnb_bc = const.tile([P, d], F32)
                nc.scalar.dma_start(lnb_bc[:], ln_b[:].to_broadcast((P, d)))

            u = const.tile([P, njt, d], F16)
            xrb = const.tile([P, nib, d], F32)
            g = [pp.tile([P, d], F32, tag=f"g{ib}", name=f"g{ib}")
                 for ib in range(nib)]
            mb = [mpool.tile([P, njt, P], F16, tag=f"mb{ib}",
                             name=f"mb{ib}") for ib in range(nib)]

            # ---- PE warmup: keep the HAM clock-gate open while the DMA
            # stream fills. Zeroed operands into g[0] as complete start/
            # stop groups ahead of the real accumulation; never read. ----
            warm = const.tile([P, d], F16)
            nc.vector.memset(warm[:], 0.0)
            for _ in range(NWARM):
                nc.tensor.matmul(g[0][:], lhsT=warm[:, 0:P], rhs=warm[:],
                                 start=True, stop=True)

            def emit_u(lo, hi):
                nc.sync.dma_start(u[:, lo:hi, :], u_d[:, lo * d:hi * d])

            def emit_m(ib, lo, hi):     # j-tiles [lo, hi) of block ib
                base = ib * njt * P
                nc.sync.dma_start(
                    mb[ib][:, lo:hi, :],
                    m_d[:, base + lo * P:base + hi * P])

            def emit_mms(ib, lo, hi):
                for jt in range(lo, hi):
                    nc.tensor.matmul(
                        g[ib][:],
                        lhsT=mb[ib][:, jt, :],
                        rhs=u[:, jt, :],
                        start=(jt == 0),
                        stop=(jt == njt - 1))

            # fine bands early (PE fed from the first microsecond), wide
            # bands later (>=4KB descriptor runs); per-block tails sized
            # so completions pitch ~1.4us >= the epilogue's DVE content.
            BANDS = [(0, 16), (16, 32), (32, 48)]
            TAIL = (48, njt)
            for bi, (lo, hi) in enumerate(BANDS):
                if bi == 0:   # split tiny head pieces: first matmul early
                    emit_u(0, 1)
                    emit_m(0, 0, 1)
                    emit_u(1, 8)
                    emit_m(0, 1, 8)
                    emit_u(8, hi)
                    emit_m(0, 8, hi)
                    emit_m(1, 0, 8)
                    emit_m(1, 8, hi)
                    for ib in range(2, nib):
                        emit_m(ib, 0, hi)
                else:
                    emit_u(lo, hi)
                    for ib in range(nib):
                        emit_m(ib, lo, hi)
                if bi == 1:   # xrb rides mid-stream, due by 1st epilogue
                    nc.sync.dma_start(xrb[:], xrb_d[:])
                for ib in range(nib):
                    emit_mms(ib, lo, hi)

            emit_u(TAIL[0], TAIL[1])
            for ib in range(nib):
                emit_m(ib, TAIL[0], TAIL[1])

            for ib in range(nib):
                emit_mms(ib, TAIL[0], TAIL[1])

                # ---- epilogue: y = G/WSCALE + xrb, then LayerNorm ----
                y = work.tile([P, d], F32, tag=f"y{ib}", bufs=1,
                              name=f"y{ib}")
                nc.vector.scalar_tensor_tensor(
                    y[:], in0=g[ib][:], scalar=1.0 / WSCALE,
                    in1=xrb[:, ib, :], op0=ALU.mult, op1=ALU.add)

                # LN stats: bn_stats halves -> bn_aggr [mean, var];
                # rstd = rsqrt(var + eps) on ACT; bias = -mean * rstd.
                st6 = work.tile([P, 6], F32, tag=f"st6_{ib}", bufs=1,
                                name=f"st6_{ib}")
                nc.vector.bn_stats(st6[:], y[:])
                mv = work.tile([P, 2], F32, tag=f"mv{ib}", bufs=1,
                               name=f"mv{ib}")
                nc.vector.bn_aggr(mv[:], st6[:])
                sc = work.tile([P, 3], F32, tag=f"sc{ib}", bufs=1,
                               name=f"sc{ib}")
                rstd, bln, stdv = sc[:, 0:1], sc[:, 1:2], sc[:, 2:3]
                nc.scalar.activation(stdv, mv[:, 1:2], AF.Sqrt,
                                     bias=eps_t[:], scale=1.0)
                nc.vector.reciprocal(rstd, stdv)
                nc.vector.scalar_tensor_tensor(
                    bln, in0=mv[:, 0:1], scalar=-1.0, in1=rstd,
                    op0=ALU.mult, op1=ALU.mult)

                yn = work.tile([P, d], F32, tag=f"yn{ib}", bufs=1,
                               name=f"yn{ib}")
                if ib == nib - 1 and not ln_affine:
                    # last block is the exposed tail: normalize halves on
                    # ACT and DVE concurrently, each feeding its store on
                    # a DIFFERENT queue so the two stores overlap
                    nc.scalar.activation(yn[:, 0:d // 2], y[:, 0:d // 2],
                                         AF.Identity, bias=bln, scale=rstd)
                    nc.vector.tensor_scalar(
                        yn[:, d // 2:d], y[:, d // 2:d], rstd, bln,
                        op0=ALU.mult, op1=ALU.add)
                    nc.scalar.dma_start(out[ts(ib, P), 0:d // 2],
                                        yn[:, 0:d // 2])
                    nc.sync.dma_start(out[ts(ib, P), d // 2:d],
                                      yn[:, d // 2:d])
                    continue
                # normalize on ACT: keeps per-chain DVE content below the
                # tail pitch so chains never queue up behind each other
                nc.scalar.activation(yn[:], y[:], AF.Identity,
                                     bias=bln, scale=rstd)
                res = yn
                if ln_affine:
                    t1 = work.tile([P, d], F32, tag=f"t1_{ib}", bufs=1,
                                   name=f"t1_{ib}")
                    nc.vector.tensor_mul(t1[:], yn[:], lns_bc[:])
                    t2 = work.tile([P, d], F32, tag=f"t2_{ib}", bufs=1,
                                   name=f"t2_{ib}")
                    nc.vector.tensor_add(t2[:], t1[:], lnb_bc[:])
                    res = t2
                # scalar queue: stores must never block the M stream
                nc.scalar.dma_start(out[ts(ib, P), :], res[:])

    nc.compile()
    return nc


_cache = {}


def _get_program(ln_affine):
    if ln_affine not in _cache:
        _cache[ln_affine] = build_program(ln_affine=ln_affine)
    return _cache[ln_affine]


def _pack(a, blocks, row_len):
    """[blocks*128, row_len] -> [128, blocks*row_len] with logical row
    blk*128+p at (p, blk*row_len)."""
    return np.ascontiguousarray(
        a.reshape(blocks, P, row_len).transpose(1, 0, 2).reshape(
            P, blocks * row_len))


def prepare_inputs(node_features, mobility_matrix, W_in, b_in, W_out, b_out,
                   ln_scale, ln_bias):
    x = np.asarray(node_features, dtype=np.float32)
    m = np.asarray(mobility_matrix, dtype=np.float32)
    w_in = np.asarray(W_in, dtype=np.float64)
    b_in_ = np.asarray(b_in, dtype=np.float64)
    w_out = np.asarray(W_out, dtype=np.float64)
    b_out_ = np.asarray(b_out, dtype=np.float64)
    lns = np.asarray(ln_scale, dtype=np.float32)
    lnb = np.asarray(ln_bias, dtype=np.float32)

    w_c = w_in @ w_out
    bias_c = (b_in_ @ w_out + b_out_).astype(np.float32)
    ln_affine = not (np.all(lns == 1.0) and np.all(lnb == 0.0))

    # exact threshold mask + host normalization, premultiplied by WSCALE
    colsum = m.sum(axis=0, dtype=np.float64)
    mm = np.where(m > (THR * (colsum + EPS))[None, :].astype(np.float32),
                  m, np.float32(0.0))
    wsum = mm.sum(axis=0, dtype=np.float64)
    col_scale = (WSCALE / (wsum + EPS * (colsum + EPS))).astype(np.float32)
    mh = (mm * col_scale[None, :]).astype(np.float16)
    del mm

    u16 = (x.astype(np.float64) @ w_c).astype(np.float16)
    u_pk = _pack(u16, N // P, D)

    s = N // NCORES
    nib = s // P
    in_maps = []
    for c in range(NCORES):
        # ib-major pack: per partition [ib][jt][128]
        m_pk = np.concatenate(
            [_pack(mh[:, c * s + ib * P:c * s + (ib + 1) * P], N // P, P)
             for ib in range(nib)], axis=1)
        in_maps.append({
            "m_pk": np.ascontiguousarray(m_pk),
            "u_pk": u_pk,
            "xrb": _pack(x[c * s:(c + 1) * s] + bias_c, s // P, D),
            "ln_s": lns.reshape(1, D),
            "ln_b": lnb.reshape(1, D),
        })
    return in_maps, ln_affine


def run(in_maps, ln_affine, **kwargs):
    nc = _get_program(ln_affine)
    return run_bass_kernel_spmd(nc, in_maps, core_ids=list(range(NCORES)),
                                **kwargs)


def kernel(**inputs) -> np.ndarray:
    in_maps, ln_affine = prepare_inputs(**inputs)
    res = run(in_maps, ln_affine)
    return np.concatenate([res.results[c]["out_shard"]
                           for c in range(NCORES)], axis=0)
